# revision 39
# baseline (speedup 1.0000x reference)
"""Trainium2 Bass kernel for nn_BPR_76665166234050 (3-hop LightGCN + BPR loss).

Strategy (8 NeuronCores, SPMD single program):
- Destinations sharded across cores; each core owns all edges into its shard,
  so per-hop segment sums are exact per core (no partial-sum all-reduce).
- Per hop: dma_gather source rows (int16 indices, tables chunked at 25088
  rows), build narrow one-hot*val lhsT matrices on DVE via broadcast-AP
  tensor_tensor ops, segment-sum on the PE into PSUM with 32-aligned window
  matmuls, drain to HBM.
- AllGather (ncfw) rebuilds full tables between dependent hops; hop order
  g1u,g1i,g2i,g2u,g3u,g3i lets every AllGather overlap the next hop's compute.
- Tail: BPR batch via quad/pair gathers + masks, self-distillation norms on
  the local shard; per-core partial sums are combined on the host.
"""
import sys
sys.path.insert(0, "/opt/trn_rl_repo")
import numpy as np


def _rup(x, m):
    return (x + m - 1) // m * m


class CFG:
    def __init__(self, user=100000, item=50000, d=64, e=3200000, b=16384,
                 ncores=8, chunk=25088, wwin=32):
        self.USER, self.ITEM, self.D, self.E, self.B = user, item, d, e, b
        self.NC = ncores
        self.CHUNK = chunk
        self.W = wwin
        self.UPAD = _rup(user, 128 * ncores)
        self.IPAD = _rup(item, 128 * ncores)
        # chunk must divide padded sizes evenly-ish; just cover with ceil
        self.UCH = (self.UPAD + chunk - 1) // chunk
        self.ICH = (self.IPAD + chunk - 1) // chunk
        self.USH = self.UPAD // ncores
        self.ISH = self.IPAD // ncores
        self.UBLK = self.USH // 128
        self.IBLK = self.ISH // 128
        self.BSH = b // ncores           # batch entries per core
        assert b % (16 * ncores) == 0


def _prep_direction(cfg, dst, src, val, sh_rows, nblk, nsch):
    """Token schedule for one direction (dest-sharded, per-core arrays).

    Returns (meta, per_core) where meta is static structure shared by all
    cores and per_core holds idx16/winval/windst arrays for each core.
    """
    NC, CHUNK, W = cfg.NC, cfg.CHUNK, cfg.W
    core = dst // sh_rows
    blk = (dst % sh_rows) // 128
    dloc = dst % 128
    sch = src // CHUNK
    sloc = (src % CHUNK).astype(np.int16)

    order = np.lexsort((dloc, sch, blk, core))
    core, blk, dloc, sch, sloc, val = (a[order] for a in
                                       (core, blk, dloc, sch, sloc, val))

    # counts per (core, blk, sch)
    key = (core * nblk + blk) * nsch + sch
    counts = np.bincount(key, minlength=NC * nblk * nsch).reshape(NC, nblk, nsch)
    caps = np.maximum(_rup(counts.max(axis=0), 128), 128)   # [nblk, nsch]

    offs = np.zeros((nblk, nsch), np.int64)
    t = 0
    for b in range(nblk):
        for c in range(nsch):
            offs[b, c] = t
            t += caps[b, c]
    T = t
    NCHT = T // 128

    # position of each edge in its core's token slab
    seg_sorted = key  # already sorted
    seg_start = np.zeros(NC * nblk * nsch + 1, np.int64)
    np.cumsum(np.bincount(seg_sorted, minlength=NC * nblk * nsch),
              out=seg_start[1:])
    rank = np.arange(len(dst)) - seg_start[seg_sorted]
    pos = offs[blk, sch] + rank

    # padding slots get idx=-1: the gather ucode trims trailing negative
    # indices, so per-core padding at the tail of each (blk, sch) window is
    # skipped entirely (no descriptors, no HBM traffic).
    idx_all = np.full((NC, T), -1, np.int16)
    val_all = np.zeros((NC, T), np.float32)
    dloc_all = np.zeros((NC, T), np.int16)
    idx_all[core, pos] = sloc
    val_all[core, pos] = val
    dloc_all[core, pos] = dloc
    cnts = counts  # [NC, nblk, nsch] true per-core counts per gather

    # per-K-chunk dest range across all cores
    cid = pos // 128
    lo = np.full(NCHT, 255, np.int64)
    hi = np.zeros(NCHT, np.int64)
    np.minimum.at(lo, cid, dloc)
    np.maximum.at(hi, cid, dloc)
    lo[lo == 255] = 0

    # windows per block: (local chunk idx, base); ensure 32-group coverage
    blocks = []
    nw = 0
    for b in range(nblk):
        c0 = offs[b, 0] // 128
        cend = (offs[b, 0] + caps[b].sum()) // 128
        # group-major window order: each 32-row psum group opens (start)
        # and closes (stop) before the next one, keeping accumulation
        # groups disjoint in time.
        wins, gstart, gstop = [], [], []
        for g in range(128 // W):
            glist = [cj - c0 for cj in range(c0, cend)
                     if lo[cj] // W <= g <= hi[cj] // W]
            if not glist:
                glist = [0]  # dummy coverage window (mask all-zero)
            for i, cj in enumerate(glist):
                wins.append((cj, g * W))
                gstart.append(i == 0)
                gstop.append(i == len(glist) - 1)
        blocks.append(dict(
            c0=c0, nch=cend - c0, woff=nw, wins=wins,
            gstart=gstart, gstop=gstop,
            goffs=[(int(offs[b, c]), int(caps[b, c])) for c in range(nsch)],
        ))
        nw += len(wins)

    # per-core device arrays
    per_core = []
    for k in range(NC):
        vw = val_all[k].reshape(NCHT, 128).T            # [128, NCHT]
        dw = dloc_all[k].astype(np.float32).reshape(NCHT, 128).T
        winval = np.zeros((128, nw), np.float32)
        windst = np.zeros((128, nw), np.float32)
        for bmeta in blocks:
            for w, (cj, base) in enumerate(bmeta["wins"]):
                gw = bmeta["woff"] + w
                winval[:, gw] = vw[:, bmeta["c0"] + cj]
                windst[:, gw] = dw[:, bmeta["c0"] + cj] - base
        iw = idx_all[k].reshape(T // 16, 16).T          # [16, T/16]
        idx16 = np.tile(iw, (8, 1))                     # [128, T/16]
        # per-gather true counts, in hop() iteration order
        gcnt = np.array([cnts[k, b, c] for b in range(nblk)
                         for c in range(nsch) if caps[b, c] > 0], np.int32)
        per_core.append(dict(idx=idx16, wval=winval, wdst=windst, gcnt=gcnt))

    meta = dict(T=T, NW=nw, blocks=blocks, caps=caps, offs=offs, nsch=nsch,
                nblk=nblk)
    return meta, per_core


def _pack_winf(meta, pc):
    """Pack [wval_blk | wdst_blk] contiguously per block -> [128, 2*NW]."""
    winf = np.zeros((128, 2 * meta["NW"]), np.float32)
    for bm in meta["blocks"]:
        o, n = bm["woff"], len(bm["wins"])
        winf[:, 2 * o:2 * o + n] = pc["wval"][:, o:o + n]
        winf[:, 2 * o + n:2 * o + 2 * n] = pc["wdst"][:, o:o + n]
    return winf


def _wrap_shard(tbl_pad, k, sh_rows):
    """[sh_rows, D] shard -> [128, (sh_rows/128)*D] wrapped for SBUF."""
    s = tbl_pad[k * sh_rows:(k + 1) * sh_rows]
    nb = sh_rows // 128
    d = s.shape[1]
    return s.reshape(nb, 128, d).transpose(1, 0, 2).reshape(128, nb * d).copy()


def _wrap_vec(vec_pad, k, sh_rows):
    s = vec_pad[k * sh_rows:(k + 1) * sh_rows]
    nb = sh_rows // 128
    return s.reshape(nb, 128).T.copy()


def _wrap_idx(ix, n):
    """flat int indices -> [128, n/16] int16 gather layout."""
    w = ix.astype(np.int16).reshape(n // 16, 16).T
    return np.tile(w, (8, 1))


def build_program(cfg, mu, mi, with_tail=True):
    """Build the Bass/Tile program (fused 4-spmm formulation).

    Q = A^T Xu; [P|S] = A [Xi|Q]; Zu = Xu/2+P/3+S/4; [R|W] = A^T [P|Zu];
    gcn_i = Xi + W; Zi = Xi/2+Q/3+R/4; gcn_u = Xu + A Zi.
    """
    import concourse.bass as bass
    import concourse.bacc as bacc
    import concourse.tile as tile
    from concourse import mybir

    D, W, NC = cfg.D, cfg.W, cfg.NC
    f32, i16 = mybir.dt.float32, mybir.dt.int16
    AOT = mybir.AluOpType

    nc = bacc.Bacc("TRN2", target_bir_lowering=False, debug=False,
                   num_devices=NC, num_swdge_queues=4)
    qrr = [0]  # round-robin SWDGE queue so gathers use all 4 Q7 core pairs

    def next_q():
        q = qrr[0]
        qrr[0] = (q + 1) % 4
        return q

    # ---- I/O ----
    uemb = nc.dram_tensor("uemb", [cfg.UPAD, D], f32, kind="ExternalInput")
    # uemb_shh / iemb_shh hold 0.5 * the wrapped shard (pre-halved on host)
    uemb_shh = nc.dram_tensor("uemb_shh", [128, cfg.UBLK * D], f32, kind="ExternalInput")
    iemb_sh = nc.dram_tensor("iemb_sh", [128, cfg.IBLK * D], f32, kind="ExternalInput")
    iemb_shh = nc.dram_tensor("iemb_shh", [128, cfg.IBLK * D], f32, kind="ExternalInput")
    oldu_sh = nc.dram_tensor("oldu_sh", [128, cfg.UBLK * D], f32, kind="ExternalInput")
    oldi_sh = nc.dram_tensor("oldi_sh", [128, cfg.IBLK * D], f32, kind="ExternalInput")
    nu_sh = nc.dram_tensor("nu_sh", [128, cfg.UBLK], f32, kind="ExternalInput")
    ni_sh = nc.dram_tensor("ni_sh", [128, cfg.IBLK], f32, kind="ExternalInput")
    idx_u = nc.dram_tensor("idx_u", [128, mu["T"] // 16], i16, kind="ExternalInput")
    idx_i = nc.dram_tensor("idx_i", [128, mi["T"] // 16], i16, kind="ExternalInput")
    # per-block packed [wval | wdst] so each block needs one window-info DMA
    winf_u = nc.dram_tensor("winf_u", [128, 2 * mu["NW"]], f32, kind="ExternalInput")
    winf_i = nc.dram_tensor("winf_i", [128, 2 * mi["NW"]], f32, kind="ExternalInput")
    ngu = sum(1 for b in mu["blocks"] for _, c in b["goffs"] if c > 0)
    ngi = sum(1 for b in mi["blocks"] for _, c in b["goffs"] if c > 0)
    i32 = mybir.dt.int32
    cnt_d = nc.dram_tensor("cnt", [128, ngu + ngi], i32, kind="ExternalInput")
    iota_in = nc.dram_tensor("iota", [128, W], f32, kind="ExternalInput")
    BSH = cfg.BSH
    bidx_u = nc.dram_tensor("bidx_u", [128, BSH // 16], i16, kind="ExternalInput")
    bidx_i = nc.dram_tensor("bidx_i", [128, BSH // 16], i16, kind="ExternalInput")
    bidx_j = nc.dram_tensor("bidx_j", [128, BSH // 16], i16, kind="ExternalInput")
    # 8 masks: 4 user quarters, 2 item_i halves, 2 item_j halves
    bmask = nc.dram_tensor("bmask", [128, (BSH // 128) * 8], f32, kind="ExternalInput")
    ones_in = nc.dram_tensor("ones", [128, 1], f32, kind="ExternalInput")
    out_d = nc.dram_tensor("out", [4], f32, kind="ExternalOutput")

    # ---- internal DRAM: AG buffers ----
    def ag_pair(name, sh, full, w):
        a = nc.dram_tensor(f"agin_{name}", [sh, w * D], f32, kind="Internal")
        o = nc.dram_tensor(f"agout_{name}", [full, w * D], f32,
                           kind="Internal", addr_space="Shared")
        return a, o

    agin_xiq, agout_xiq = ag_pair("xiq", cfg.ISH, cfg.IPAD, 2)
    agin_pzu, agout_pzu = ag_pair("pzu", cfg.USH, cfg.UPAD, 2)
    agin_zi, agout_zi = ag_pair("zi", cfg.ISH, cfg.IPAD, 1)
    agin_gcu, agout_gcu = ag_pair("gcu", cfg.USH, cfg.UPAD, 1)
    agin_gci, agout_gci = ag_pair("gci", cfg.ISH, cfg.IPAD, 1)

    maxT_u = max(sum(c for _, c in b["goffs"]) for b in mu["blocks"])
    maxT_i = max(sum(c for _, c in b["goffs"]) for b in mi["blocks"])
    maxT = max(maxT_u, maxT_i)
    maxW = max(max(len(b["wins"]) for b in mu["blocks"]),
               max(len(b["wins"]) for b in mi["blocks"]))

    with tile.TileContext(nc) as tc:
        with (
            tc.tile_pool(name="persist", bufs=1) as pp,
            tc.tile_pool(name="io", bufs=3) as iop,
            tc.tile_pool(name="gath", bufs=3) as gp,
            tc.tile_pool(name="gath2", bufs=2) as gp2,
            tc.tile_pool(name="lhs", bufs=2) as lp,
            tc.tile_pool(name="drain", bufs=3) as dp,
            tc.tile_pool(name="psum", bufs=3, space="PSUM") as psp,
            tc.tile_pool(name="psumt", bufs=1, space="PSUM") as pst,
            tc.tile_pool(name="tail", bufs=1) as tp,
        ):
            q_sh = pp.tile([128, cfg.IBLK, D], f32, tag="qsh")  # Q shard
            nu_t = pp.tile([128, cfg.UBLK], f32, tag="nu")
            ni_t = pp.tile([128, cfg.IBLK], f32, tag="ni")
            iota_t = pp.tile([128, W], f32, tag="iota")
            part_t = pp.tile([128, 4], f32, tag="part")
            cnt_t = pp.tile([128, ngu + ngi], i32, tag="cnt")
            nc.vector.memset(part_t[:], 0.0)
            nc.sync.dma_start(iota_t[:], iota_in.ap())
            nc.sync.dma_start(cnt_t[:], cnt_d.ap())
            nc.sync.dma_start(nu_t[:], nu_sh.ap())
            nc.sync.dma_start(ni_t[:], ni_sh.ap())
            # stage raw Xi shard into left half of agin_xiq (DRAM -> DRAM)
            nc.sync.dma_start(
                agin_xiq.ap().rearrange("(b p) c -> p b c", p=128)[:, :, 0:D],
                iemb_sh.ap().rearrange("p (b d) -> p b d", d=D))

            def load_half(src_d, b, tag):
                t = iop.tile([128, D], f32, tag=tag)
                nc.sync.dma_start(t[:], src_d.ap()[:, b * D:(b + 1) * D])
                return t

            def hop(meta, idx_d, winf_d, cbase, src_buf, src_rows, width,
                    finish):
                """One spmm hop. src_buf: DRAM [src_rows, width*D]."""
                src_ap = src_buf.ap()
                gi = 0
                for b, bm in enumerate(meta["blocks"]):
                    Tb = sum(c for _, c in bm["goffs"])
                    nwb = len(bm["wins"])
                    off0 = bm["goffs"][0][0]
                    idx_t = iop.tile([128, maxT // 16], i16, tag="idx")
                    wvd_t = iop.tile([128, 2 * maxW], f32, tag="wvd")
                    nc.sync.dma_start(
                        idx_t[:, :Tb // 16],
                        idx_d.ap()[:, off0 // 16:(off0 + Tb) // 16])
                    nc.sync.dma_start(
                        wvd_t[:, :2 * nwb],
                        winf_d.ap()[:, 2 * bm["woff"]:2 * bm["woff"] + 2 * nwb])
                    wv_t = wvd_t[:, 0:nwb]
                    wd_t = wvd_t[:, nwb:2 * nwb]

                    gpool = gp if width == 1 else gp2
                    g_t = gpool.tile([128, maxT // 128, width * D], f32,
                                     tag=f"g{width}")
                    for c, (off, cap) in enumerate(bm["goffs"]):
                        if cap == 0:
                            continue
                        rel = off - off0
                        lo_row = c * cfg.CHUNK
                        hi_row = min(lo_row + cfg.CHUNK, src_rows)
                        creg = nc.gpsimd.alloc_register()
                        nc.gpsimd.reg_load(
                            creg, cnt_t[0:1, cbase + gi:cbase + gi + 1])
                        gi += 1
                        nc.gpsimd.dma_gather(
                            g_t[:, rel // 128:(rel + cap) // 128, :],
                            src_ap[lo_row:hi_row, :],
                            idx_t[:, rel // 16:(rel + cap) // 16],
                            num_idxs=cap,
                            num_idxs_reg=creg,
                            elem_size=width * D,
                            single_packet=False,
                            queue_num=next_q(),
                        )

                    l_t = lp.tile([128, maxW, W], f32, tag="l")
                    dst_b = wd_t.broadcast_to([128, nwb, W])
                    iota_b = iota_t[:].rearrange(
                        "p (c w) -> p c w", c=1).broadcast_to([128, nwb, W])
                    val_b = wv_t.broadcast_to([128, nwb, W])
                    nc.vector.tensor_tensor(
                        l_t[:, :nwb, :], dst_b, iota_b, AOT.is_equal)
                    nc.vector.tensor_tensor(
                        l_t[:, :nwb, :], l_t[:, :nwb, :], val_b, AOT.mult)

                    ps_t = psp.tile([128, width * D], f32, tag=f"ps{width}")
                    for w, (cj, base) in enumerate(bm["wins"]):
                        nc.tensor.matmul(
                            ps_t[base:base + W, :],
                            l_t[:, w, :],
                            g_t[:, cj, :],
                            start=bm["gstart"][w],
                            stop=bm["gstop"][w],
                            tile_position=(0, base),
                        )
                    finish(b, ps_t)

            def allgather(ag_in, ag_out):
                nc.gpsimd.collective_compute(
                    "AllGather", mybir.AluOpType.bypass,
                    replica_groups=[list(range(NC))],
                    ins=[ag_in.ap()], outs=[ag_out.ap()],
                )

            def self_loss_blk(gcn_blk, old_d, n_t, b, col):
                """part[:, col] += ||gcn_blk - old_blk||_2 * n[:, b]"""
                old_t = iop.tile([128, D], f32, tag="old")
                nc.sync.dma_start(old_t[:],
                                  old_d.ap()[:, b * D:(b + 1) * D])
                nc.vector.tensor_tensor(old_t[:], gcn_blk, old_t[:],
                                        AOT.subtract)
                nc.vector.tensor_tensor(old_t[:], old_t[:], old_t[:],
                                        AOT.mult)
                rs = dp.tile([128, 1], f32, tag="rs")
                nc.vector.tensor_reduce(rs[:], old_t[:],
                                        mybir.AxisListType.X, AOT.add)
                nc.scalar.activation(rs[:], rs[:],
                                     mybir.ActivationFunctionType.Sqrt)
                nc.vector.tensor_tensor(rs[:], rs[:], n_t[:, b:b + 1],
                                        AOT.mult)
                nc.vector.tensor_tensor(part_t[:, col:col + 1],
                                        part_t[:, col:col + 1], rs[:],
                                        AOT.add)

            U = (mu, idx_u, winf_u, 0)
            I = (mi, idx_i, winf_i, ngu)

            # one-time memset of gather slots: padding slots trimmed by the
            # ucode are never written, so clear any initial NaN bit patterns
            for _ in range(3):
                z = gp.tile([128, maxT // 128, D], f32, tag="g1")
                nc.vector.memset(z[:], 0.0)
            for _ in range(2):
                z = gp2.tile([128, maxT // 128, 2 * D], f32, tag="g2")
                nc.vector.memset(z[:], 0.0)

            # ---- spmm1: Q = A^T Xu ----
            def fin1(b, ps):
                nc.scalar.copy(q_sh[:, b, :], ps[:, 0:D])
                nc.sync.dma_start(
                    agin_xiq.ap()[b * 128:(b + 1) * 128, D:2 * D],
                    q_sh[:, b, :])

            hop(*I, uemb, cfg.UPAD, 1, fin1)
            allgather(agin_xiq, agout_xiq)

            # ---- spmm2: [P|S] = A [Xi|Q]; Zu = Xu/2 + P/3 + S/4 ----
            def fin2(b, ps):
                dr = dp.tile([128, 2 * D], f32, tag="dr2")
                nc.scalar.copy(dr[:, 0:D], ps[:, 0:D])
                xuh = load_half(uemb_shh, b, "xub")
                tmp = dp.tile([128, D], f32, tag="tmp")
                nc.vector.scalar_tensor_tensor(
                    tmp[:], ps[:, D:2 * D], 0.25, xuh[:],
                    AOT.mult, AOT.add)
                nc.vector.scalar_tensor_tensor(
                    dr[:, D:2 * D], ps[:, 0:D], 1.0 / 3.0, tmp[:],
                    AOT.mult, AOT.add)
                nc.sync.dma_start(
                    agin_pzu.ap()[b * 128:(b + 1) * 128, :], dr[:])

            hop(*U, agout_xiq, cfg.IPAD, 2, fin2)
            allgather(agin_pzu, agout_pzu)

            # ---- spmm3: [R|W] = A^T [P|Zu]; gcn_i = Xi + W;
            #      Zi = Xi/2 + Q/3 + R/4 ----
            def fin3(b, ps):
                xih = load_half(iemb_shh, b, "xib")
                dr = dp.tile([128, D], f32, tag="dr3")
                tmp = dp.tile([128, D], f32, tag="tmp")
                nc.vector.scalar_tensor_tensor(
                    tmp[:], ps[:, 0:D], 0.25, xih[:],
                    AOT.mult, AOT.add)
                nc.vector.scalar_tensor_tensor(
                    dr[:], q_sh[:, b, :], 1.0 / 3.0, tmp[:],
                    AOT.mult, AOT.add)
                nc.sync.dma_start(
                    agin_zi.ap()[b * 128:(b + 1) * 128, :], dr[:])
                gt = dp.tile([128, D], f32, tag="gci")
                nc.vector.scalar_tensor_tensor(
                    gt[:], xih[:], 2.0, ps[:, D:2 * D],
                    AOT.mult, AOT.add)
                nc.sync.dma_start(
                    agin_gci.ap()[b * 128:(b + 1) * 128, :], gt[:])
                if with_tail:
                    self_loss_blk(gt[:], oldi_sh, ni_t, b, 3)

            hop(*I, agout_pzu, cfg.UPAD, 2, fin3)
            allgather(agin_zi, agout_zi)
            allgather(agin_gci, agout_gci)

            # ---- spmm4: gcn_u = Xu + A Zi ----
            def fin4(b, ps):
                xuh = load_half(uemb_shh, b, "xub")
                gt = dp.tile([128, D], f32, tag="gcu")
                nc.vector.scalar_tensor_tensor(
                    gt[:], xuh[:], 2.0, ps[:, 0:D],
                    AOT.mult, AOT.add)
                nc.sync.dma_start(
                    agin_gcu.ap()[b * 128:(b + 1) * 128, :], gt[:])
                if with_tail:
                    self_loss_blk(gt[:], oldu_sh, nu_t, b, 2)

            hop(*U, agout_zi, cfg.IPAD, 1, fin4)
            allgather(agin_gcu, agout_gcu)

            if with_tail:
                # BPR batch: gathers from AG'd gcn tables
                BS = BSH // 128  # free-dim slots
                mask_t = tp.tile([128, 8 * BS], f32, tag="bmask")
                nc.sync.dma_start(mask_t[:], bmask.ap())

                def batch_rows(src_full, rows_full, group, bidx_d, mask_lo, ngrp,
                               tag):
                    """gather fused rows [128, BS, group*D]; mask-select -> [128,BS,D]"""
                    gt_full = tp.tile([128, BS * 4 * D], f32, tag="bgshare")
                    gt = gt_full[:, :BS * group * D].rearrange(
                        "p (s gd) -> p s gd", gd=group * D)
                    bix_t = tp.tile([128, BSH // 16], i16, tag=f"bx{tag}")
                    nc.sync.dma_start(bix_t[:], bidx_d.ap())
                    src2 = src_full.ap().rearrange("(a g) d -> a (g d)", g=group)
                    nc.gpsimd.dma_gather(
                        gt[:], src2, bix_t[:],
                        num_idxs=BSH, num_idxs_reg=BSH, elem_size=group * D,
                        single_packet=False, queue_num=next_q())
                    rt = tp.tile([128, BS, D], f32, tag=f"br{tag}")
                    tmp = tp.tile([128, BS, D], f32, tag="btshare")
                    for q in range(ngrp):
                        m_b = mask_t[:, (mask_lo + q) * BS:(mask_lo + q + 1) * BS]\
                            .broadcast_to([128, BS, D])
                        dstt = rt if q == 0 else tmp
                        nc.vector.tensor_tensor(
                            dstt[:], gt[:, :, q * D:(q + 1) * D], m_b, AOT.mult)
                        if q > 0:
                            nc.vector.tensor_tensor(rt[:], rt[:], tmp[:], AOT.add)
                    return rt

                u_t = batch_rows(agout_gcu, cfg.UPAD, 4, bidx_u, 0, 4, "u")
                ii_t = batch_rows(agout_gci, cfg.IPAD, 2, bidx_i, 4, 2, "i")
                ij_t = batch_rows(agout_gci, cfg.IPAD, 2, bidx_j, 6, 2, "j")

                pr = tp.tile([128, BS, D], f32, tag="btshare")
                pi = tp.tile([128, BS], f32, tag="pi")
                pj = tp.tile([128, BS], f32, tag="pj")
                nc.vector.tensor_tensor(pr[:], u_t[:], ii_t[:], AOT.mult)
                nc.vector.tensor_reduce(pi[:], pr[:], mybir.AxisListType.X, AOT.add)
                nc.vector.tensor_tensor(pr[:], u_t[:], ij_t[:], AOT.mult)
                nc.vector.tensor_reduce(pj[:], pr[:], mybir.AxisListType.X, AOT.add)
                nc.vector.tensor_tensor(pi[:], pi[:], pj[:], AOT.subtract)
                # -log_sigmoid(x) summed: part0 = sum(ln(sigmoid(x))), negated on host
                bt = tp.tile([128, BS], f32, tag="bt2")
                nc.scalar.activation(bt[:], pi[:],
                                     mybir.ActivationFunctionType.Sigmoid)
                nc.scalar.activation(bt[:], bt[:],
                                     mybir.ActivationFunctionType.Ln,
                                     accum_out=part_t[:, 0:1])

                # reg = sum over batch of rowsum(u^2+ii^2+ij^2)
                rg = tp.tile([128, BS], f32, tag="rg")
                rgt = tp.tile([128, BS], f32, tag="rgt")
                nc.vector.tensor_tensor(pr[:], u_t[:], u_t[:], AOT.mult)
                nc.vector.tensor_reduce(rg[:], pr[:], mybir.AxisListType.X, AOT.add)
                nc.vector.tensor_tensor(pr[:], ii_t[:], ii_t[:], AOT.mult)
                nc.vector.tensor_reduce(rgt[:], pr[:], mybir.AxisListType.X, AOT.add)
                nc.vector.tensor_tensor(rg[:], rg[:], rgt[:], AOT.add)
                nc.vector.tensor_tensor(pr[:], ij_t[:], ij_t[:], AOT.mult)
                nc.vector.tensor_reduce(rgt[:], pr[:], mybir.AxisListType.X, AOT.add)
                nc.vector.tensor_tensor(rg[:], rg[:], rgt[:], AOT.add)
                nc.vector.tensor_reduce(part_t[:, 1:2], rg[:],
                                        mybir.AxisListType.X, AOT.add)

            # cross-partition sum of the 4 partial columns via ones-matmul
            ones_t = tp.tile([128, 1], f32, tag="ones")
            nc.sync.dma_start(ones_t[:], ones_in.ap())
            ps4 = pst.tile([4, 1], f32, tag="ps4")
            nc.tensor.matmul(ps4[:], part_t[:], ones_t[:],
                             start=True, stop=True)
            out_t = tp.tile([4, 1], f32, tag="out4")
            nc.scalar.copy(out_t[:], ps4[:])
            nc.sync.dma_start(out_d.ap().rearrange("(a b) -> a b", b=1),
                              out_t[:])

    nc.compile()
    return nc


def _preprocess(cfg, inputs, pad_gather=False):
    """Host prep: returns (mu, mi, in_maps)."""
    user = np.asarray(inputs["user"]).astype(np.int64)
    item_i = np.asarray(inputs["item_i"]).astype(np.int64)
    item_j = np.asarray(inputs["item_j"]).astype(np.int64)
    edge_u = np.asarray(inputs["edge_u"]).astype(np.int64)
    edge_i = np.asarray(inputs["edge_i"]).astype(np.int64)
    edge_val = np.asarray(inputs["edge_val"]).astype(np.float32)
    user_emb = np.asarray(inputs["user_emb"]).astype(np.float32)
    item_emb = np.asarray(inputs["item_emb"]).astype(np.float32)
    old_U = np.asarray(inputs["old_U_emb"]).astype(np.float32)
    old_I = np.asarray(inputs["old_I_emb"]).astype(np.float32)
    n_U = np.asarray(inputs["n_U"]).astype(np.float32)
    n_I = np.asarray(inputs["n_I"]).astype(np.float32)

    D = cfg.D

    def pad_rows(a, n):
        out = np.zeros((n,) + a.shape[1:], a.dtype)
        out[:len(a)] = a
        return out

    uemb_p = pad_rows(user_emb, cfg.UPAD)
    iemb_p = pad_rows(item_emb, cfg.IPAD)
    oldu_p = pad_rows(old_U, cfg.UPAD)
    oldi_p = pad_rows(old_I, cfg.IPAD)
    nu_p = pad_rows(n_U, cfg.UPAD)
    ni_p = pad_rows(n_I, cfg.IPAD)

    mu, pc_u = _prep_direction(cfg, edge_u, edge_i, edge_val,
                               cfg.USH, cfg.UBLK, cfg.ICH)
    mi, pc_i = _prep_direction(cfg, edge_i, edge_u, edge_val,
                               cfg.ISH, cfg.IBLK, cfg.UCH)
    if pad_gather:
        # sim-friendly mode: gather row 0 for padding instead of trimming
        # trailing -1 indices (the sim NaN-poisons unwritten tile regions)
        for meta, pcs in ((mu, pc_u), (mi, pc_i)):
            caps_list = np.array([c for b in meta["blocks"]
                                  for _, c in b["goffs"] if c > 0], np.int32)
            for pc in pcs:
                pc["idx"] = np.maximum(pc["idx"], 0)
                pc["gcnt"] = caps_list.copy()

    iota = np.broadcast_to(np.arange(cfg.W, dtype=np.float32),
                           (128, cfg.W)).copy()
    ones = np.ones((128, 1), np.float32)

    in_maps = []
    BSH, BS = cfg.BSH, cfg.BSH // 128
    for k in range(cfg.NC):
        bs = slice(k * BSH, (k + 1) * BSH)
        bu, bi, bj = user[bs], item_i[bs], item_j[bs]
        masks = np.zeros((128, 8 * BS), np.float32)
        for q in range(4):
            m = (bu % 4 == q).astype(np.float32).reshape(BS, 128).T
            masks[:, q * BS:(q + 1) * BS] = m
        for q in range(2):
            m = (bi % 2 == q).astype(np.float32).reshape(BS, 128).T
            masks[:, (4 + q) * BS:(5 + q) * BS] = m
            m = (bj % 2 == q).astype(np.float32).reshape(BS, 128).T
            masks[:, (6 + q) * BS:(7 + q) * BS] = m
        in_maps.append({
            "uemb": uemb_p,
            "uemb_shh": 0.5 * _wrap_shard(uemb_p, k, cfg.USH),
            "iemb_sh": _wrap_shard(iemb_p, k, cfg.ISH),
            "iemb_shh": 0.5 * _wrap_shard(iemb_p, k, cfg.ISH),
            "oldu_sh": _wrap_shard(oldu_p, k, cfg.USH),
            "oldi_sh": _wrap_shard(oldi_p, k, cfg.ISH),
            "nu_sh": _wrap_vec(nu_p, k, cfg.USH),
            "ni_sh": _wrap_vec(ni_p, k, cfg.ISH),
            "idx_u": pc_u[k]["idx"], "winf_u": _pack_winf(mu, pc_u[k]),
            "idx_i": pc_i[k]["idx"], "winf_i": _pack_winf(mi, pc_i[k]),
            "cnt": np.broadcast_to(
                np.concatenate([pc_u[k]["gcnt"], pc_i[k]["gcnt"]]),
                (128, len(pc_u[k]["gcnt"]) + len(pc_i[k]["gcnt"]))).copy(),
            "iota": iota, "ones": ones,
            "bidx_u": _wrap_idx(bu // 4, BSH),
            "bidx_i": _wrap_idx(bi // 2, BSH),
            "bidx_j": _wrap_idx(bj // 2, BSH),
            "bmask": masks,
        })
    return mu, mi, in_maps


def run(cfg, inputs, trace=False, use_sim=False, **bkw):
    from concourse import bass_utils
    mu, mi, in_maps = _preprocess(cfg, inputs, pad_gather=use_sim)
    nc = build_program(cfg, mu, mi, **bkw)
    if use_sim:
        from concourse.bass_interp import MultiCoreSim
        sim = MultiCoreSim(nc, num_cores=cfg.NC, trace=False)
        cores = [sim.cores[i] for i in sorted(sim.cores)]
        for k, core in enumerate(cores):
            for name, arr in in_maps[k].items():
                core.tensor(name)[:] = arr
        sim.simulate(check_with_hw=False)

        class R:
            results = [{"out": np.array(core.tensor("out"))}
                       for core in cores]
        res = R()
    else:
        res = bass_utils.run_bass_kernel_spmd(
            nc, in_maps, core_ids=list(range(cfg.NC)), trace=trace)
    parts = np.stack([res.results[k]["out"] for k in range(cfg.NC)])
    tot = parts.sum(axis=0)          # [ln_sig_sum, reg_sum, ud_sum, id_sum]
    loss_bpr = -tot[0] / cfg.B + 1e-4 * tot[1] / cfg.B
    loss_self = tot[2] / cfg.USER + tot[3] / cfg.ITEM
    out = np.array([loss_bpr, 100.0 * loss_self, 1.0, 1.0], np.float32)
    return out, res


def kernel(**inputs):
    cfg = CFG()
    out, _ = run(cfg, inputs)
    return out

